# revision 35
# baseline (speedup 1.0000x reference)
"""Causal linear attention (Katharopoulos et al.) Trainium2 Bass kernel.

Problem: B=4, L=2048, H=8, D=64, f32.
  Q' = elu(Q)+1, K' = elu(K)+1
  out[b,l,h,:] = (sum_{i<=l} (Q'[l].K'[i]) V[i]) / (Q'[l].cumsum(K')[l] + eps)

Sharding: 8 cores, core c <- batch b=c//2, head-quad hq=c%2 (4 heads).
Per-core problem: q,k,v [2048, 4*64] -> o [2048, 4*64].

Host staging (all f16): Q pre-transposed per head-pair (qT [256, 2048],
rows = pair*128 + head_in_pair*64 + d), K natural [2048, 256], V
chunk-position-major [128, T*256]. ~5MB HBM traffic per core.

Per chunk (C=128) the PE runs 14 matmuls:
  2 K-transposes (head-pair batched), 4 A^T (row-tiled head pairs, the
  row-group-64 pair in its own PSUM bank), 4 Y_intra, 2 Y_inter
  (block-diagonal S, pair batched), 2 S-updates (pair batched,
  persistent PSUM accumulation).
Feature map via elu(x)+1 == max(x+1, min(exp(x), 1)) (e^x >= 1+x):
  ex=exp(x) [Scalar, direct from load], t1=x+1 [GpSimd],
  u=min(ex,1) [Vector TS 4x], fp=max(t1,u) [Vector TT 2x].
"""

import numpy as np
import ml_dtypes

import concourse.bass as bass
import concourse.bacc as bacc
import concourse.tile as tile
from concourse import mybir
from concourse.bass_utils import run_bass_kernel_spmd
from concourse.masks import make_identity, make_upper_triangular


F32 = mybir.dt.float32
F16 = mybir.dt.float16
B, L, H, D = 4, 2048, 8, 64
N_CORES = 8
HPC = 4          # heads per core
W = HPC * D      # 256
C = 128          # sequence chunk
T = L // C       # 16 chunks
AluOp = mybir.AluOpType
Act = mybir.ActivationFunctionType

# load groups: (first_chunk, n_chunks); small first groups so the PE
# starts as early as possible
GROUPS = [(0, 1), (1, 1), (2, 2), (4, 2), (6, 2), (8, 4), (12, 4)]
NG = len(GROUPS)


def _ap(t, offset_elems, dims):
    base = t[:] if not isinstance(t, bass.AP) else t
    return bass.AP(tensor=base.tensor, offset=base.offset + offset_elems, ap=dims)


def build_bass() -> bass.Bass:
    nc = bacc.Bacc(None, target_bir_lowering=False, debug=False)
    qt_d = nc.dram_tensor("qT", [2 * C, L], F16, kind="ExternalInput")
    k_d = nc.dram_tensor("k", [L, W], F16, kind="ExternalInput")
    # v staged chunk-position-major with the ones column pre-filled by the
    # host: v_d[p, ((t*4 + h)*65 + d)] = V[t*C+p, h, d] for d<64, 1.0 at d=64
    v_d = nc.dram_tensor("v", [C, T * 4 * 65], F16, kind="ExternalInput")
    o_d = nc.dram_tensor("o", [L, W], F32, kind="ExternalOutput")

    group_of = {}
    for gi, (c0, n) in enumerate(GROUPS):
        for tt in range(n):
            group_of[c0 + tt] = (gi, tt)

    with tile.TileContext(nc) as tc:
        with (
            tc.tile_pool(name="consts", bufs=1) as consts,
            tc.tile_pool(name="state", bufs=1) as state,
            tc.tile_pool(name="ldq", bufs=4) as ldq,
            tc.tile_pool(name="ldv", bufs=4) as ldv,
            tc.tile_pool(name="fmw", bufs=3) as fmw,
            tc.tile_pool(name="fmp", bufs=3) as fmp,
            tc.tile_pool(name="tws", bufs=3) as tws,
            tc.tile_pool(name="work", bufs=3) as work,
            tc.tile_pool(name="outp", bufs=3) as outp,
            tc.tile_pool(name="kt_ps", bufs=1, space="PSUM") as kt_pool,
            tc.tile_pool(name="at_ps", bufs=1, space="PSUM") as at_pool,
            tc.tile_pool(name="y1_ps", bufs=2, space="PSUM") as y1_pool,
            tc.tile_pool(name="s_ps", bufs=1, space="PSUM") as s_pool,
        ):
            ident = consts.tile([128, 128], F16)
            make_identity(nc, ident)
            # dummy exp so the ACT_TABLE_LOAD happens during the first DMA
            # wait instead of on the first real feature-map op
            scratch = consts.tile([128, 1], F16)
            nc.scalar.activation(out=scratch, in_=ident[:, 0:1], func=Act.Exp)
            mask1 = consts.tile([128, 128], F16)
            make_upper_triangular(nc, mask1, val=1.0, diag=True)

            # persistent S accumulator [128(2h x 64d), pair, 130(2h x 65)]
            s_ps = s_pool.tile([128, 2, 130], F32)
            nc.vector.memset(s_ps, 0.0)
            # block-diagonal S in SBUF f16 (zeros persist outside diag blocks)
            s2b = state.tile([128, 2, 130], F16)
            nc.gpsimd.memset(s2b, 0.0)

            qks = {}   # group -> load tile [128, n, 4, 128]
            exs = {}   # group -> exp tile
            t1s = {}   # group -> x+1 tile
            fp4 = {}   # group -> fm'd tile [128, n, 4, 128]
            v14 = {}   # group -> [128, n, 4, 65]
            kts = {}   # chunk -> kt sbuf [128, 2, 128]
            atms = {}  # chunk -> atm [128, 4, 128]
            ots = {}   # even chunk -> ot tile [128, 2, 4, 64]

            def emit_load(gi):
                c0, n = GROUPS[gi]
                r0 = c0 * C
                # slot layout [:, tt, 0:2, :] = qT pairs, [:, tt, 2:4, :] = k
                qk = ldq.tile([128, n, 4, 128], F16, name=f"qk_{gi}", tag="qk")
                nc.sync.dma_start(
                    out=qk[:, :, 2:4, :].rearrange("p t s c -> p t (s c)"),
                    in_=k_d[r0 : r0 + n * C, :].rearrange("(t p) w -> p t w", p=C),
                )
                for p in range(2):
                    nc.sync.dma_start(
                        out=qk[:, :, p, :],
                        in_=qt_d[p * C : (p + 1) * C, r0 : r0 + n * C].rearrange(
                            "p (t c) -> p t c", c=C),
                    )
                v1 = ldv.tile([128, n, 4, 65], F16, name=f"v1_{gi}", tag="v1")
                nc.gpsimd.dma_start(
                    out=v1[:].rearrange("p t h d -> p (t h d)"),
                    in_=v_d[:, c0 * 260 : (c0 + n) * 260],
                )
                v14[gi] = v1
                qks[gi] = qk

            def emit_fm(gi, part=None):
                """part: None = whole group; (lo, hi) = chunk-slot half;
                'k0'/'q0' = k/q columns of a 1-chunk group (fast first chunk)."""
                c0, n = GROUPS[gi]
                if gi not in exs:
                    exs[gi] = fmw.tile([128, n, 4, 128], F16, name=f"ex_{gi}",
                                       tag="ex")
                    t1s[gi] = fmw.tile([128, n, 4, 128], F16, name=f"t1_{gi}",
                                       tag="t1")
                    fp4[gi] = fmp.tile([128, n, 4, 128], F16, name=f"fp_{gi}",
                                       tag="fp")
                qk, ex, t1, fp = qks[gi], exs[gi], t1s[gi], fp4[gi]
                if part == 'k0':
                    sl = lambda t_: t_[:, :, 2:4, :]
                elif part == 'q0':
                    sl = lambda t_: t_[:, :, 0:2, :]
                elif part is None:
                    sl = lambda t_: t_[:].rearrange("p t s c -> p (t s c)")
                else:
                    lo, hi = part
                    sl = lambda t_: t_[:, lo:hi, :, :]
                nc.scalar.activation(out=sl(ex), in_=sl(qk), func=Act.Exp)
                u = fmw.tile([128, n, 4, 128], F16, name=f"u_{gi}_{part}", tag="u")
                nc.vector.tensor_scalar_min(out=sl(u), in0=sl(ex), scalar1=1.0)
                nc.vector.tensor_scalar_add(out=sl(t1), in0=sl(qk), scalar1=1.0)
                nc.vector.tensor_max(out=sl(fp), in0=sl(t1), in1=sl(u))

            def emit_transpose(t):
                gi, tt = group_of[t]
                fp = fp4[gi]
                kt_ps = kt_pool.tile([128, 2, 128], F32, name=f"ktps_{t}", tag="ktps")
                for p in range(2):
                    nc.tensor.matmul(
                        out=kt_ps[:, p, :],
                        lhsT=fp[:, tt, 2 + p, :], rhs=ident)
                kt = tws.tile([128, 2, 128], F16, name=f"kt_{t}", tag="kt")
                nc.scalar.copy(out=kt, in_=kt_ps)
                kts[t] = kt

            def emit_at(t):
                gi, tt = group_of[t]
                fp = fp4[gi]
                # row-group 0 and row-group 64 matmuls must land in DIFFERENT
                # psum banks (same-bank mixed row groups wedge the PE): one
                # 2-bank tile, head-in-pair i in bank i (f32 offset i*512)
                at2 = at_pool.tile([128, 1024], F32, name=f"atps_{t}", tag="atps")
                for p in range(2):
                    for i in range(2):
                        nc.tensor.matmul(
                            out=_ap(at2, 512 * i + 128 * p,
                                    [at2[:].ap[0], [1, 128]]),
                            lhsT=kts[t][64 * i : 64 * i + 64, p, :],
                            rhs=fp[64 * i : 64 * i + 64, tt, p, :])
                atm = work.tile([128, 4, 128], F16, name=f"atm_{t}", tag="atm")
                nc.vector.tensor_mul(
                    out=_ap(atm, 0, [atm[:].ap[0], [128, 2], [256, 2], [1, 128]]),
                    in0=_ap(at2, 0, [at2[:].ap[0], [512, 2], [128, 2], [1, 128]]),
                    in1=_ap(mask1, 0, [mask1[:].ap[0], [0, 2], [0, 2], [1, 128]]))
                atms[t] = atm
                del kts[t]

            y1s = {}

            def emit_tail(t):
                gi, tt = group_of[t]
                fp = fp4[gi]
                v1 = v14[gi]
                # y1 for a chunk PAIR: [128, 2, 512] f32 = 2 psum banks, one
                # bank (512 f32, 260 used) per chunk. One accumulation group
                # per bank: first matmul opens it, per-element has_written
                # bits let later start=False matmuls overwrite fresh columns.
                if t % 2 == 0:
                    y1s[t] = y1_pool.tile([128, 2, 512], F32, name=f"y1_{t}",
                                          tag="y1")
                y1p = y1s[t - t % 2]
                j = t % 2
                for h in range(4):
                    nc.tensor.matmul(
                        out=_ap(y1p, 512 * j + 65 * h, [y1p[:].ap[0], [1, 65]]),
                        lhsT=atms[t][:, h, :],
                        rhs=v1[:, tt, h, :],
                        start=(h == 0), stop=(t == 0 and h == 3))
                if t > 0:
                    for p in range(2):
                        nc.tensor.matmul(
                            out=_ap(y1p, 512 * j + 130 * p,
                                    [y1p[:].ap[0], [1, 130]]),
                            lhsT=fp[:, tt, p, :],
                            rhs=s2b[:, p, :],
                            start=False, stop=(p == 1))
                if t < T - 1:
                    for p in range(2):
                        nc.tensor.matmul(
                            out=s_ps[:, p, :],
                            lhsT=fp[:, tt, 2 + p, :],
                            rhs=v1[:, tt, 2 * p : 2 * p + 2, :].rearrange(
                                "p a b -> p (a b)"),
                            start=False, stop=(t == T - 2),
                            skip_group_check=True)
                    # refresh block-diagonal S (diag blocks only; zeros persist)
                    nc.scalar.copy(out=s2b[0:64, :, 0:65], in_=s_ps[0:64, :, 0:65])
                    nc.scalar.copy(
                        out=s2b[64:128, :, 65:130], in_=s_ps[64:128, :, 65:130])

                del atms[t]
                if t % 2 == 0:
                    return
                # normalize + store the pair in single wide ops
                zr = outp.tile([128, 2, 4], F32, name=f"zr_{t}", tag="zr")
                nc.vector.reciprocal(
                    out=zr,
                    in_=_ap(y1p, 64, [y1p[:].ap[0], [512, 2], [65, 4]]))
                ot = outp.tile([128, 2, 4, 64], F32, name=f"ot_{t}", tag="ot")
                nc.vector.tensor_mul(
                    out=ot,
                    in0=_ap(y1p, 0, [y1p[:].ap[0], [512, 2], [65, 4], [1, 64]]),
                    in1=_ap(zr, 0, [zr[:].ap[0], [4, 2], [1, 4], [0, 64]]))
                nc.sync.dma_start(
                    out=o_d[(t - 1) * C : (t + 1) * C, :].rearrange(
                        "(c p) (h d) -> p c h d", p=C, d=64),
                    in_=ot)
                del y1s[t - 1]

            # post-compute emissions per iteration: loads + fm bursts (fm of
            # group g+1 is emitted near the end of group g, split into halves
            # for the big groups so vector-queue bursts stay small)
            post = {
                0: [("load", 2, None), ("fm", 1, None)],
                1: [("load", 3, None), ("fm", 2, None)],
                2: [("load", 4, None)],
                3: [("fm", 3, None)],
                4: [("load", 5, None)],
                5: [("fm", 4, None)],
                6: [("load", 6, None)],
                7: [("fm", 5, (0, 2))],
                8: [("fm", 5, (2, 4))],
                11: [("fm", 6, (0, 2))],
                12: [("fm", 6, (2, 4))],
            }

            # prologue: chunk-0 k columns get their own fm so the first
            # transpose isn't gated on the q-side load
            emit_load(0)
            emit_fm(0, part='k0')
            emit_fm(0, part='q0')
            emit_load(1)
            for t in range(T + 3):
                if t < T:
                    emit_transpose(t)
                if 1 <= t <= T:
                    emit_at(t - 1)
                if 3 <= t:
                    emit_tail(t - 3)
                for kind, gi, part in post.get(t, []):
                    if kind == "load":
                        emit_load(gi)
                    else:
                        emit_fm(gi, part)
    nc.compile()
    return nc


_nc_cache = None


def _get_nc():
    global _nc_cache
    if _nc_cache is None:
        _nc_cache = build_bass()
    return _nc_cache


def make_in_maps(queries, keys, values):
    queries = np.asarray(queries)
    keys = np.asarray(keys)
    values = np.asarray(values)
    in_maps = []
    for c in range(N_CORES):
        b, hq = c // 2, c % 2
        hs = slice(hq * HPC, (hq + 1) * HPC)
        q = queries[b, :, hs, :]               # [L, 4, 64]
        qT = np.ascontiguousarray(
            np.moveaxis(q, 0, -1).reshape(2 * C, L)).astype(np.float16)
        k = np.ascontiguousarray(keys[b, :, hs, :]).reshape(L, W).astype(np.float16)
        v4 = values[b, :, hs, :].reshape(T, C, 4, 64).transpose(1, 0, 2, 3)
        v1 = np.concatenate(
            [v4, np.ones((C, T, 4, 1), v4.dtype)], axis=3)  # ones col at d=64
        v = np.ascontiguousarray(v1.reshape(C, T * 4 * 65)).astype(np.float16)
        in_maps.append({"qT": qT, "k": k, "v": v})
    return in_maps


def kernel(queries: np.ndarray, keys: np.ndarray, values: np.ndarray) -> np.ndarray:
    nc = _get_nc()
    in_maps = make_in_maps(queries, keys, values)
    res = run_bass_kernel_spmd(nc, in_maps, core_ids=list(range(N_CORES))).results
    out = np.empty((B, L, H, D), dtype=np.float32)
    for c in range(N_CORES):
        b, hq = c // 2, c % 2
        out[b, :, hq * HPC : (hq + 1) * HPC, :] = res[c]["o"].reshape(L, HPC, D)
    return out


# revision 47
# speedup vs baseline: 1.1547x; 1.1547x over previous
"""Causal linear attention (Katharopoulos et al.) Trainium2 Bass kernel.

Problem: B=4, L=2048, H=8, D=64, f32.
  Q' = elu(Q)+1, K' = elu(K)+1
  out[b,l,h,:] = (sum_{i<=l} (Q'[l].K'[i]) V[i]) / (Q'[l].cumsum(K')[l] + eps)

Sharding: 8 cores, core c <- batch b=c//2, head-quad hq=c%2 (4 heads).
Per-core problem: q,k,v [2048, 4*64] -> o [2048, 4*64].

Host staging (all f16): Q pre-transposed per head-pair (qT [256, 2048],
rows = pair*128 + head_in_pair*64 + d), K natural [2048, 256], V
chunk-position-major [128, T*256]. ~5MB HBM traffic per core.

Per chunk (C=128) the PE runs 14 matmuls:
  2 K-transposes (head-pair batched), 4 A^T (row-tiled head pairs, the
  row-group-64 pair in its own PSUM bank), 4 Y_intra, 2 Y_inter
  (block-diagonal S, pair batched), 2 S-updates (pair batched,
  persistent PSUM accumulation).
Feature map via elu(x)+1 == max(x+1, min(exp(x), 1)) (e^x >= 1+x):
  ex=exp(x) [Scalar, direct from load], t1=x+1 [GpSimd],
  u=min(ex,1) [Vector TS 4x], fp=max(t1,u) [Vector TT 2x].
"""

import numpy as np

import concourse.bass as bass
import concourse.bacc as bacc
import concourse.tile as tile
from concourse import mybir
from concourse.bass_utils import run_bass_kernel_spmd
from concourse.masks import make_identity, make_upper_triangular


F32 = mybir.dt.float32
F16 = mybir.dt.float16
B, L, H, D = 4, 2048, 8, 64
N_CORES = 8
HPC = 4          # heads per core
W = HPC * D      # 256
C = 128          # sequence chunk
T = L // C       # 16 chunks
AluOp = mybir.AluOpType
Act = mybir.ActivationFunctionType

# load groups: (first_chunk, n_chunks); small first groups so the PE
# starts as early as possible
GROUPS = [(0, 1), (1, 1), (2, 2), (4, 4), (8, 4), (12, 4)]
NG = len(GROUPS)


def _ap(t, offset_elems, dims):
    base = t[:] if not isinstance(t, bass.AP) else t
    return bass.AP(tensor=base.tensor, offset=base.offset + offset_elems, ap=dims)


def build_bass() -> bass.Bass:
    nc = bacc.Bacc(None, target_bir_lowering=False, debug=False)
    qt_d = nc.dram_tensor("qT", [2 * C, L], F16, kind="ExternalInput")
    k_d = nc.dram_tensor("k", [L, W], F16, kind="ExternalInput")
    # v staged chunk-position-major with the ones column pre-filled by the
    # host: v_d[p, ((t*4 + h)*65 + d)] = V[t*C+p, h, d] for d<64, 1.0 at d=64
    v_d = nc.dram_tensor("v", [C, T * 4 * 65], F16, kind="ExternalInput")
    # output chunk-major too: o_d[p, t*W + h*64 + d] = out[t*C+p, h, d];
    # the host untangles. Store DMAs become fully contiguous [128, 512].
    o_d = nc.dram_tensor("o", [C, T * W], F32, kind="ExternalOutput")

    group_of = {}
    for gi, (c0, n) in enumerate(GROUPS):
        for tt in range(n):
            group_of[c0 + tt] = (gi, tt)

    with tile.TileContext(nc) as tc:
        with (
            tc.tile_pool(name="consts", bufs=1) as consts,
            tc.tile_pool(name="state", bufs=1) as state,
            tc.tile_pool(name="ldq", bufs=3) as ldq,
            tc.tile_pool(name="ldv", bufs=3) as ldv,
            tc.tile_pool(name="fmw", bufs=2) as fmw,
            tc.tile_pool(name="fmp", bufs=3) as fmp,
            tc.tile_pool(name="tws", bufs=3) as tws,
            tc.tile_pool(name="work", bufs=3) as work,
            tc.tile_pool(name="outp", bufs=3) as outp,
            tc.tile_pool(name="kt_ps", bufs=1, space="PSUM") as kt_pool,
            tc.tile_pool(name="at_ps", bufs=1, space="PSUM") as at_pool,
            tc.tile_pool(name="y1_ps", bufs=2, space="PSUM") as y1_pool,
            tc.tile_pool(name="s_ps", bufs=1, space="PSUM") as s_pool,
        ):
            ident = consts.tile([128, 128], F16)
            make_identity(nc, ident)
            # dummy exp so the ACT_TABLE_LOAD happens during the first DMA
            # wait instead of on the first real feature-map op
            scratch = consts.tile([128, 1], F16)
            nc.scalar.activation(out=scratch, in_=ident[:, 0:1], func=Act.Exp)
            # warm-up matmuls: keep the PE busy through the initial load wait
            # so the HAM clock gate is already at 8/8 when real work starts
            warm = y1_pool.tile([128, 2, 512], F32, name="warm", tag="y1")
            for _ in range(20):
                nc.tensor.matmul(out=warm[:, 0, 0:128], lhsT=ident, rhs=ident)
            mask1 = consts.tile([128, 128], F16)
            make_upper_triangular(nc, mask1, val=1.0, diag=True)

            # persistent S accumulator [128(2h x 64d), pair, 130(2h x 65)]
            s_ps = s_pool.tile([128, 2, 130], F32)
            nc.vector.memset(s_ps, 0.0)
            # block-diagonal S in SBUF f16 (zeros persist outside diag blocks)
            s2b = state.tile([128, 2, 130], F16)
            nc.gpsimd.memset(s2b, 0.0)

            qks = {}   # group -> load tile [128, n, 4, 128]
            exs = {}   # group -> exp tile
            t1s = {}   # group -> x+1 tile
            fp4 = {}   # group -> fm'd tile [128, n, 4, 128]
            v14 = {}   # group -> [128, n, 4, 65]
            kts = {}   # chunk -> kt sbuf [128, 2, 128]
            atms = {}  # chunk -> atm [128, 4, 128]
            ots = {}   # even chunk -> ot tile [128, 2, 4, 64]

            def emit_load(gi):
                c0, n = GROUPS[gi]
                r0 = c0 * C
                # slot layout [:, tt, 0:2, :] = qT pairs, [:, tt, 2:4, :] = k
                qk = ldq.tile([128, n, 4, 128], F16, name=f"qk_{gi}", tag="qk")
                nc.sync.dma_start(
                    out=qk[:, :, 2:4, :].rearrange("p t s c -> p t (s c)"),
                    in_=k_d[r0 : r0 + n * C, :].rearrange("(t p) w -> p t w", p=C),
                )
                for p in range(2):
                    nc.sync.dma_start(
                        out=qk[:, :, p, :],
                        in_=qt_d[p * C : (p + 1) * C, r0 : r0 + n * C].rearrange(
                            "p (t c) -> p t c", c=C),
                    )
                v1 = ldv.tile([128, n, 4, 65], F16, name=f"v1_{gi}", tag="v1")
                nc.gpsimd.dma_start(
                    out=v1[:].rearrange("p t h d -> p (t h d)"),
                    in_=v_d[:, c0 * 260 : (c0 + n) * 260],
                )
                v14[gi] = v1
                qks[gi] = qk

            def emit_fm(gi, part=None):
                """part: None = whole group; (lo, hi) = chunk-slot half;
                'k0'/'q0' = k/q columns of a 1-chunk group (fast first chunk)."""
                c0, n = GROUPS[gi]
                if gi not in exs:
                    exs[gi] = fmw.tile([128, n, 4, 128], F16, name=f"ex_{gi}",
                                       tag="ex")
                    t1s[gi] = fmw.tile([128, n, 4, 128], F16, name=f"t1_{gi}",
                                       tag="t1")
                    fp4[gi] = fmp.tile([128, n, 4, 128], F16, name=f"fp_{gi}",
                                       tag="fp")
                qk, ex, t1, fp = qks[gi], exs[gi], t1s[gi], fp4[gi]
                if part == 'k0':
                    sl = lambda t_: t_[:, :, 2:4, :]
                elif part == 'q0':
                    sl = lambda t_: t_[:, :, 0:2, :]
                elif part is None:
                    sl = lambda t_: t_[:].rearrange("p t s c -> p (t s c)")
                else:
                    lo, hi = part
                    sl = lambda t_: t_[:, lo:hi, :, :]
                nc.scalar.activation(out=sl(ex), in_=sl(qk), func=Act.Exp)
                u = fmw.tile([128, n, 4, 128], F16, name=f"u_{gi}_{part}", tag="u")
                nc.vector.tensor_scalar_min(out=sl(u), in0=sl(ex), scalar1=1.0)
                nc.vector.tensor_scalar_add(out=sl(t1), in0=sl(qk), scalar1=1.0)
                nc.vector.tensor_max(out=sl(fp), in0=sl(t1), in1=sl(u))

            def emit_transpose(t):
                gi, tt = group_of[t]
                fp = fp4[gi]
                kt_ps = kt_pool.tile([128, 2, 128], F32, name=f"ktps_{t}", tag="ktps")
                for p in range(2):
                    nc.tensor.matmul(
                        out=kt_ps[:, p, :],
                        lhsT=fp[:, tt, 2 + p, :], rhs=ident)
                kt = tws.tile([128, 2, 128], F16, name=f"kt_{t}", tag="kt")
                nc.any.tensor_copy(out=kt, in_=kt_ps)
                kts[t] = kt

            def emit_at(t):
                gi, tt = group_of[t]
                fp = fp4[gi]
                # row-group 0 and row-group 64 matmuls must land in DIFFERENT
                # psum banks (same-bank mixed row groups wedge the PE): one
                # 2-bank tile, head-in-pair i in bank i (f32 offset i*512)
                at2 = at_pool.tile([128, 1024], F32, name=f"atps_{t}", tag="atps")
                for p in range(2):
                    for i in range(2):
                        nc.tensor.matmul(
                            out=_ap(at2, 512 * i + 128 * p,
                                    [at2[:].ap[0], [1, 128]]),
                            lhsT=kts[t][64 * i : 64 * i + 64, p, :],
                            rhs=fp[64 * i : 64 * i + 64, tt, p, :])
                atm = work.tile([128, 4, 128], F16, name=f"atm_{t}", tag="atm")
                nc.vector.tensor_mul(
                    out=_ap(atm, 0, [atm[:].ap[0], [128, 2], [256, 2], [1, 128]]),
                    in0=_ap(at2, 0, [at2[:].ap[0], [512, 2], [128, 2], [1, 128]]),
                    in1=_ap(mask1, 0, [mask1[:].ap[0], [0, 2], [0, 2], [1, 128]]))
                atms[t] = atm
                del kts[t]

            y1s = {}

            def emit_tail(t):
                gi, tt = group_of[t]
                fp = fp4[gi]
                v1 = v14[gi]
                # y1 for a chunk PAIR: [128, 2, 512] f32 = 2 psum banks, one
                # bank (512 f32, 260 used) per chunk. One accumulation group
                # per bank: first matmul opens it, per-element has_written
                # bits let later start=False matmuls overwrite fresh columns.
                if t % 2 == 0:
                    y1s[t] = y1_pool.tile([128, 2, 512], F32, name=f"y1_{t}",
                                          tag="y1")
                y1p = y1s[t - t % 2]
                j = t % 2
                for h in range(4):
                    nc.tensor.matmul(
                        out=_ap(y1p, 512 * j + 65 * h, [y1p[:].ap[0], [1, 65]]),
                        lhsT=atms[t][:, h, :],
                        rhs=v1[:, tt, h, :],
                        start=(h == 0), stop=(t == 0 and h == 3))
                if t > 0:
                    for p in range(2):
                        nc.tensor.matmul(
                            out=_ap(y1p, 512 * j + 130 * p,
                                    [y1p[:].ap[0], [1, 130]]),
                            lhsT=fp[:, tt, p, :],
                            rhs=s2b[:, p, :],
                            start=False, stop=(p == 1))
                if t < T - 1:
                    for p in range(2):
                        nc.tensor.matmul(
                            out=s_ps[:, p, :],
                            lhsT=fp[:, tt, 2 + p, :],
                            rhs=v1[:, tt, 2 * p : 2 * p + 2, :].rearrange(
                                "p a b -> p (a b)"),
                            start=False, stop=(t == T - 2),
                            skip_group_check=True)
                    # refresh block-diagonal S (diag blocks only; zeros persist)
                    nc.any.tensor_copy(
                        out=s2b[0:64, :, 0:65], in_=s_ps[0:64, :, 0:65])
                    nc.any.tensor_copy(
                        out=s2b[64:128, :, 65:130], in_=s_ps[64:128, :, 65:130])

                del atms[t]
                if t % 2 == 0:
                    return
                # normalize + store the pair in single wide ops
                zr = outp.tile([128, 2, 4], F32, name=f"zr_{t}", tag="zr")
                nc.vector.reciprocal(
                    out=zr,
                    in_=_ap(y1p, 64, [y1p[:].ap[0], [512, 2], [65, 4]]))
                ot = outp.tile([128, 2, 4, 64], F32, name=f"ot_{t}", tag="ot")
                nc.vector.tensor_mul(
                    out=ot,
                    in0=_ap(y1p, 0, [y1p[:].ap[0], [512, 2], [65, 4], [1, 64]]),
                    in1=_ap(zr, 0, [zr[:].ap[0], [4, 2], [1, 4], [0, 64]]))
                nc.sync.dma_start(
                    out=o_d[:, (t - 1) * W : (t + 1) * W],
                    in_=ot[:].rearrange("p c h d -> p (c h d)"))
                del y1s[t - 1]

            # post-compute emissions per iteration: loads + fm bursts (fm of
            # group g+1 is emitted near the end of group g, split into halves
            # for the big groups so vector-queue bursts stay small)
            post = {
                0: [("load", 2, None), ("fm", 1, None)],
                1: [("load", 3, None), ("fm", 2, None)],
                2: [("load", 4, None)],
                3: [("fm", 3, (0, 2))],
                4: [("fm", 3, (2, 4)), ("load", 5, None)],
                7: [("fm", 4, (0, 2))],
                8: [("fm", 4, (2, 4))],
                11: [("fm", 5, (0, 2))],
                12: [("fm", 5, (2, 4))],
            }

            # prologue: chunk-0 k columns get their own fm so the first
            # transpose isn't gated on the q-side load
            emit_load(0)
            emit_fm(0, part='k0')
            emit_fm(0, part='q0')
            emit_load(1)
            for t in range(T + 3):
                if t < T:
                    emit_transpose(t)
                if 1 <= t <= T:
                    emit_at(t - 1)
                if 3 <= t:
                    emit_tail(t - 3)
                for kind, gi, part in post.get(t, []):
                    if kind == "load":
                        emit_load(gi)
                    else:
                        emit_fm(gi, part)
    nc.compile()
    return nc


_nc_cache = None


def _get_nc():
    global _nc_cache
    if _nc_cache is None:
        _nc_cache = build_bass()
    return _nc_cache


def make_in_maps(queries, keys, values):
    queries = np.asarray(queries)
    keys = np.asarray(keys)
    values = np.asarray(values)
    in_maps = []
    for c in range(N_CORES):
        b, hq = c // 2, c % 2
        hs = slice(hq * HPC, (hq + 1) * HPC)
        q = queries[b, :, hs, :]               # [L, 4, 64]
        qT = np.ascontiguousarray(
            np.moveaxis(q, 0, -1).reshape(2 * C, L)).astype(np.float16)
        k = np.ascontiguousarray(keys[b, :, hs, :]).reshape(L, W).astype(np.float16)
        v4 = values[b, :, hs, :].reshape(T, C, 4, 64).transpose(1, 0, 2, 3)
        v1 = np.concatenate(
            [v4, np.ones((C, T, 4, 1), v4.dtype)], axis=3)  # ones col at d=64
        v = np.ascontiguousarray(v1.reshape(C, T * 4 * 65)).astype(np.float16)
        in_maps.append({"qT": qT, "k": k, "v": v})
    return in_maps


def unstage_out(o):
    # o_d is chunk-major [C, T*W]; untangle to [L, HPC, D]
    return np.ascontiguousarray(
        o.reshape(C, T, HPC, D).transpose(1, 0, 2, 3).reshape(L, HPC, D))


def kernel(queries: np.ndarray, keys: np.ndarray, values: np.ndarray) -> np.ndarray:
    nc = _get_nc()
    in_maps = make_in_maps(queries, keys, values)
    res = run_bass_kernel_spmd(nc, in_maps, core_ids=list(range(N_CORES))).results
    out = np.empty((B, L, H, D), dtype=np.float32)
    for c in range(N_CORES):
        b, hq = c // 2, c % 2
        out[b, :, hq * HPC : (hq + 1) * HPC, :] = unstage_out(res[c]["o"])
    return out
